# revision 1
# baseline (speedup 1.0000x reference)
"""Dual-stream transformer block on 8 TRN2 NeuronCores.

Sharding: pure data-parallel over batch (16 batches -> 2 per core), zero
collectives. Per core, each batch of 1024 tokens flows through a
transposed-layout dataflow so every matmul contracts over the partition dim:

  x (N-layout: [tok,feat]) --LN stats--> PE-transpose --> xnT (T-layout)
  qT/kT = W^T @ xnT (T-proj), v = xn @ Wv (N-proj, lhsT = xnT)
  scoresT = k @ qT (PSUM) --ACT exp--> probsT (fp32r)
  ctxT += v.T @ probsT ; den += ones.T @ probsT (den broadcast via all-ones
  128x128 stationary); ctxT normalized by reciprocal(den)
  attT = Wo^T @ ctxT + residual (self-attn, stays in T-layout)
  cross-attn output projected back to N-layout (lhsT = ctxT slices), then
  LNf -> transpose -> MLP (gelu on ACT) -> residual -> DMA out.

All matmuls run as float32r (TF32-like, 1 cyc/row at N=512; measured rel err
~1.5e-4). V-projection biases are folded into the attention output bias on
the host (softmax rows sum to 1, so P @ (v + bv) @ Wo + bo
= (P@v)@Wo + (bv@Wo + bo)).
"""
import sys

sys.path.insert(0, "/opt/trn_rl_repo")

import numpy as np

import bass_rust
import concourse.bass as bass
import concourse.mybir as mybir
import concourse.tile as tile
from concourse.bass import ts, ds
from concourse.bass_utils import run_bass_kernel_spmd
from concourse.vector_clock import ScopedClock

dt = mybir.dt
AF = mybir.ActivationFunctionType
OP = mybir.AluOpType

H = 4
D = 512
MLP = 1024
S = 1024
B = 16
NCORES = 8
BL = B // NCORES  # batches per core
EPS = 1e-6
KD = D // 128   # 4 feature tiles
KM = MLP // 128  # 8 mlp tiles
MT = S // 128   # 8 token tiles
QN = S // 512   # 2 query chunks
INV_SQRT_HD = float(1.0 / np.sqrt(128.0))

_MAX_WAITS = int(__import__('os').environ.get('MAXW', '1'))


class CompatTileContext(tile.TileContext):
    """Works around this walrus build's 1-sync-wait-per-instruction limit:
    extra waits are hoisted into single-wait NoOps on the same engine."""

    _nop_ctr = 0

    def _hoist_waits(self, inst):
        si = inst.sync_info
        if si is None:
            return
        waits = list(si.on_wait)
        if len(waits) <= _MAX_WAITS:
            return
        if inst.engine == mybir.EngineType.Unassigned:
            return
        keep = [waits[-1]]
        for w in waits[:-1]:
            CompatTileContext._nop_ctr += 1
            nop = mybir.InstNoOp(
                name=f"waitnop-{CompatTileContext._nop_ctr}", ins=[], outs=[]
            )
            nop.engine = inst.engine
            nop.sync_info = bass_rust.SyncInfo(on_wait=[w], on_update=[])
            super()._add_instruction(nop)
        inst.sync_info = bass_rust.SyncInfo(on_wait=keep, on_update=list(si.on_update))

    def _add_instruction(self, inst):
        self._hoist_waits(inst)
        super()._add_instruction(inst)

    def _drain_and_barrier(self, tick_clock, wait_clock):
        drain_inst = self.nc.sync.drain()
        wait_clock.add_sem_waits(
            drain_inst.ins, ScopedClock({None: tick_clock.global_clock})
        )
        inst = drain_inst.ins
        si = inst.sync_info
        waits = list(si.on_wait)
        if len(waits) > _MAX_WAITS:
            inst.sync_info = bass_rust.SyncInfo(
                on_wait=waits[:_MAX_WAITS], on_update=list(si.on_update)
            )
            for w in waits[_MAX_WAITS:]:
                nop = self.nc.sync.nop()
                nop.ins.sync_info = bass_rust.SyncInfo(on_wait=[w], on_update=[])
        self.nc.all_engine_barrier()
        popped = self.nc._tile_sem_poison_stack.pop()
        assert popped is self._sem_poison
        self.nc.clear_and_free_semaphores(list(self.sems.allocated().values()))
        self.nc.all_engine_barrier()


def _build(repeat=1, **opts):
    nc = bass.Bass("TRN2", target_bir_lowering=False, debug=False, num_devices=NCORES)

    f32 = dt.float32
    f32r = dt.float32r

    def din(name, shape, d=f32):
        return nc.dram_tensor(name, shape, d, kind="ExternalInput").ap()

    x1 = din("x1", [BL, S, D])
    x2 = din("x2", [BL, S, D])
    w = {}
    for nm in ["q1", "k1", "v1", "q2", "k2", "v2", "q12", "k12", "v12", "o"]:
        w[nm] = din("W" + nm, [D, D], f32r)
    w1 = din("W1", [D, MLP], f32r)
    w2 = din("W2", [MLP, D], f32r)
    bq = {nm: din("b" + nm, [D]) for nm in ["q1", "k1", "q2", "k2", "q12", "k12"]}
    b1 = din("b1", [MLP])
    b2 = din("b2", [D])
    boe = {i: din(f"boe{i}", [D]) for i in (1, 2, 3)}
    lng = {nm: din(nm, [D]) for nm in ["ln1_g", "ln1_b", "ln2_g", "ln2_b", "lnf_g", "lnf_b"]}
    ident_d = din("ident", [128, 128])
    ones_d = din("ones_sq", [128, 128], f32r)
    out = nc.dram_tensor("out", [BL, S, D], f32, kind="ExternalOutput").ap()

    from contextlib import ExitStack

    with CompatTileContext(nc) as tc, ExitStack() as ctx:
        cst = ctx.enter_context(tc.tile_pool(name="cst", bufs=1))
        t2 = ctx.enter_context(tc.tile_pool(name="t2", bufs=6))
        big8 = ctx.enter_context(tc.tile_pool(name="big8", bufs=2))
        xsb_p = ctx.enter_context(tc.tile_pool(name="xsb", bufs=1))
        xrow = ctx.enter_context(tc.tile_pool(name="xrow", bufs=3))
        xnrow = ctx.enter_context(tc.tile_pool(name="xnrow", bufs=2))
        obuf = ctx.enter_context(tc.tile_pool(name="obuf", bufs=1))
        wk = ctx.enter_context(tc.tile_pool(name="wk", bufs=4))
        wk1 = ctx.enter_context(tc.tile_pool(name="wk1", bufs=4))
        wk2 = ctx.enter_context(tc.tile_pool(name="wk2", bufs=8))
        probs = ctx.enter_context(tc.tile_pool(name="probs", bufs=opts.get("probs_bufs", 3)))
        rrep = ctx.enter_context(tc.tile_pool(name="rrep", bufs=2))
        stp = ctx.enter_context(tc.tile_pool(name="stp", bufs=8))
        pp = ctx.enter_context(tc.tile_pool(name="pp", bufs=opts.get("pp", 2), space="PSUM"))
        ps_sc = ctx.enter_context(tc.tile_pool(name="ps_sc", bufs=opts.get("ps_sc", 3), space="PSUM"))
        ps_ctx = ctx.enter_context(tc.tile_pool(name="ps_ctx", bufs=opts.get("ps_ctx", 1), space="PSUM"))
        ps_den = ctx.enter_context(tc.tile_pool(name="ps_den", bufs=opts.get("ps_den", 1), space="PSUM"))
        ps_t = ctx.enter_context(tc.tile_pool(name="ps_t", bufs=opts.get("ps_t", 1), space="PSUM"))

        # ---- constants ----
        ident = cst.tile([128, 128], f32, tag="ident")
        nc.sync.dma_start(ident[:], ident_d)
        ones_sq = cst.tile([128, 128], f32r, tag="ones")
        nc.sync.dma_start(ones_sq[:], ones_d)
        eps_t = cst.tile([128, 1], f32, tag="eps")
        nc.vector.memset(eps_t[:], EPS)

        def col_t(name_ap, n, tag):
            t = cst.tile([128, n], f32, tag=tag)
            nc.sync.dma_start(t[:], name_ap.rearrange("(t p) -> p t", p=128))
            return t

        lnT = {nm: col_t(lng[nm], KD, "lnT_" + nm) for nm in lng}
        bqT = {nm: col_t(bq[nm], KD, "bqT_" + nm) for nm in bq}
        b1T = col_t(b1, KM, "b1T")
        boeT = {i: col_t(boe[i], KD, f"boeT{i}") for i in (1, 2)}

        def rep_t(src_ap, tag):
            t = cst.tile([128, D], f32, tag=tag)
            bc = bass.AP(tensor=src_ap.tensor, offset=src_ap.offset, ap=[[0, 128], [1, D]])
            nc.sync.dma_start(t[:], bc)
            return t

        boe3_rep = rep_t(boe[3], "boe3r")
        b2_rep = rep_t(b2, "b2r")

        # ---- helpers ----
        def load_wk(wap, k, pool=wk, n=D, tag="wk", c0=0):
            t = pool.tile([128, n], f32r, tag=tag)
            nc.sync.dma_start(t[:], wap[ts(k, 128), ds(c0, n)])
            return t

        def _transpose_out(xn_ap, gT, bT, outT, mt):
            for ft in range(KD):
                if opts.get("pt_share_pp", False):
                    pt = pp.tile([128, 128], f32, tag="pp")
                else:
                    pt = ps_t.tile([128, 128], f32, tag="pst")
                nc.tensor.transpose(pt[:], xn_ap[:, ts(ft, 128)], ident[:])
                nc.vector.tensor_scalar(
                    outT[:, ft, ts(mt, 128)], pt[:],
                    gT[:, ft:ft + 1], bT[:, ft:ft + 1],
                    op0=OP.mult, op1=OP.add,
                )

        def ln_rows_to_T(get_row, gT, bT, outT):
            """LN over rows (N-layout) then PE-transpose into outT [128,KD,S]
            with gamma/beta applied per-partition on the transposed side.
            rstd comes from a pair-batched Newton rsqrt on DVE: no ACT Sqrt,
            so the exp/gelu activation tables never get evicted by LN."""
            PAIR = 2
            for p0 in range(0, MT, PAIR):
                rows = []
                mv = stp.tile([128, PAIR, 2], f32, tag="mv2")
                for i in range(PAIR):
                    xr = get_row(p0 + i)
                    rows.append(xr)
                    if opts.get("ablate_ln", False):
                        continue
                    stats = stp.tile([128, 6], f32, tag="st6")
                    nc.vector.bn_stats(stats[:], xr)
                    nc.vector.bn_aggr(mv[:, i, :], stats[:])
                if opts.get("ablate_ln", False):
                    for i in range(PAIR):
                        _transpose_out(rows[i], gT, bT, outT, p0 + i)
                    continue
                ve = stp.tile([128, PAIR], f32, tag="veps")
                nc.vector.tensor_scalar_add(ve[:], mv[:, :, 1], eps_t[:, 0:1])
                y = stp.tile([128, PAIR], f32, tag="nwy")
                t = stp.tile([128, PAIR], f32, tag="nwt")
                nc.vector.memset(y[:], 0.7)
                for _ in range(5):
                    nc.vector.tensor_mul(t[:], y[:], y[:])
                    nc.vector.tensor_mul(t[:], t[:], ve[:])
                    nc.vector.tensor_scalar(
                        t[:], t[:], -0.5, 1.5, op0=OP.mult, op1=OP.add
                    )
                    nc.vector.tensor_mul(y[:], y[:], t[:])
                for i in range(PAIR):
                    xn = xnrow.tile([128, D], f32, tag="xn")
                    nc.vector.tensor_scalar(
                        xn[:], rows[i], mv[:, i, 0:1], y[:, i:i + 1],
                        op0=OP.subtract, op1=OP.mult,
                    )
                    _transpose_out(xn[:], gT, bT, outT, p0 + i)

        def ln_dram_to_T(src2d, gT, bT, outT):
            def get_row(mt):
                xr = xrow.tile([128, D], f32, tag="xr")
                nc.sync.dma_start(xr[:], src2d[ts(mt, 128), :])
                return xr[:]
            ln_rows_to_T(get_row, gT, bT, outT)

        def proj_T(wap, bias_col, rhsT, outT):
            """outT[:,m,n] = W^T @ rhsT + bias (per-partition)."""
            wt = [load_wk(wap, k) for k in range(KD)]
            for m in range(KD):
                for n in range(QN):
                    ps = pp.tile([128, 512], f32, tag="pp")
                    for k in range(KD):
                        nc.tensor.matmul(
                            ps[:], wt[k][:, ts(m, 128)], rhsT[:, k, ts(n, 512)],
                            start=(k == 0), stop=(k == KD - 1),
                        )
                    if opts.get("qk_on_act", True):
                        nc.scalar.activation(
                            outT[:, m, ts(n, 512)], ps[:], AF.Identity,
                            bias=bias_col[:, m:m + 1],
                        )
                    else:
                        nc.vector.tensor_scalar_add(
                            outT[:, m, ts(n, 512)], ps[:], bias_col[:, m:m + 1]
                        )

        def proj_v(wap, lhsT, outv):
            """outv[:,mt,:] = (x @ Wv) natural layout; bias folded on host."""
            wt = [load_wk(wap, k) for k in range(KD)]
            for mt in range(MT):
                ps = pp.tile([128, 512], f32, tag="pp")
                for k in range(KD):
                    nc.tensor.matmul(
                        ps[:], lhsT[:, k, ts(mt, 128)], wt[k][:],
                        start=(k == 0), stop=(k == KD - 1),
                    )
                nc.vector.tensor_copy(outv[:, mt, :], ps[:])

        def attention(qT, kT, v, ctxT):
            if opts.get("ablate_att", False):
                return
            for h in range(H):
                for qn in range(QN):
                    qsl = ds(qn * 512, 512)
                    ctx_ps = ps_ctx.tile([128, 512], f32, tag="psc")
                    den_ps = ps_den.tile([128, 512], f32, tag="psd")
                    for kt in range(MT):
                        sc = ps_sc.tile([128, 512], f32, tag="pss")
                        nc.tensor.matmul(
                            sc[:], kT[:, h, ts(kt, 128)], qT[:, h, qsl],
                            start=True, stop=True,
                        )
                        pt = probs.tile([128, 512], f32r, tag="pb")
                        nc.scalar.activation(pt[:], sc[:], AF.Exp, scale=INV_SQRT_HD)
                        nc.tensor.matmul(
                            ctx_ps[:], v[:, kt, ts(h, 128)], pt[:],
                            start=(kt == 0), stop=(kt == MT - 1),
                        )
                        nc.tensor.matmul(
                            den_ps[:], ones_sq[:], pt[:],
                            start=(kt == 0), stop=(kt == MT - 1),
                        )
                    rr = rrep.tile([128, 512], f32, tag="rr")
                    nc.vector.reciprocal(rr[:], den_ps[:])
                    nc.vector.tensor_mul(ctxT[:, h, qsl], ctx_ps[:], rr[:])

        def att_out_T(ctxT, boe_col, resT, outT):
            """outT = Wo^T @ ctxT + boe + resT (self-attn, T-layout)."""
            wt = [load_wk(w["o"], k) for k in range(KD)]
            for m in range(KD):
                for n in range(QN):
                    ps = pp.tile([128, 512], f32, tag="pp")
                    for k in range(KD):
                        nc.tensor.matmul(
                            ps[:], wt[k][:, ts(m, 128)], ctxT[:, k, ts(n, 512)],
                            start=(k == 0), stop=(k == KD - 1),
                        )
                    nc.vector.scalar_tensor_tensor(
                        out=outT[:, m, ts(n, 512)], in0=ps[:],
                        scalar=boe_col[:, m:m + 1], in1=resT[:, m, ts(n, 512)],
                        op0=OP.add, op1=OP.add,
                    )

        def att_out_N(ctxT, x1_2d, x_sb):
            """x_sb[:,mt,:] = ctx3 @ Wo + boe3 + x1 (natural layout)."""
            wt = [load_wk(w["o"], k) for k in range(KD)]
            for mt in range(MT):
                ps = pp.tile([128, 512], f32, tag="pp")
                for k in range(KD):
                    nc.tensor.matmul(
                        ps[:], ctxT[:, k, ts(mt, 128)], wt[k][:],
                        start=(k == 0), stop=(k == KD - 1),
                    )
                xr = xrow.tile([128, D], f32, tag="xr")
                nc.sync.dma_start(xr[:], x1_2d[ts(mt, 128), :])
                nc.vector.tensor_add(x_sb[:, mt, :], ps[:], boe3_rep[:])
                nc.vector.tensor_add(x_sb[:, mt, :], x_sb[:, mt, :], xr[:])

        # ================= per-batch program =================
        for b in [bb for _ in range(repeat) for bb in range(BL)]:
            x1nT = t2.tile([128, KD, S], f32r, tag="t2")
            ln_dram_to_T(x1[b], lnT["ln1_g"], lnT["ln1_b"], x1nT)

            q1T = t2.tile([128, KD, S], f32r, tag="t2")
            k1T = t2.tile([128, KD, S], f32r, tag="t2")
            v1 = big8.tile([128, MT, D], f32r, tag="big8")
            proj_T(w["q1"], bqT["q1"], x1nT, q1T)
            proj_T(w["k1"], bqT["k1"], x1nT, k1T)
            proj_v(w["v1"], x1nT, v1)

            if opts.get("ln2_early", True):
                x2nT = t2.tile([128, KD, S], f32r, tag="t2")
                ln_dram_to_T(x2[b], lnT["ln2_g"], lnT["ln2_b"], x2nT)

            ctx1T = t2.tile([128, KD, S], f32r, tag="t2")
            attention(q1T, k1T, v1, ctx1T)
            if opts.get("ablate_att", False):
                ctx1T = q1T
            src1T = t2.tile([128, KD, S], f32r, tag="t2")
            att_out_T(ctx1T, boeT[1], x1nT, src1T)

            if not opts.get("ln2_early", True):
                x2nT = t2.tile([128, KD, S], f32r, tag="t2")
                ln_dram_to_T(x2[b], lnT["ln2_g"], lnT["ln2_b"], x2nT)
            q2T = t2.tile([128, KD, S], f32r, tag="t2")
            k2T = t2.tile([128, KD, S], f32r, tag="t2")
            v2 = big8.tile([128, MT, D], f32r, tag="big8")
            proj_T(w["q2"], bqT["q2"], x2nT, q2T)
            proj_T(w["k2"], bqT["k2"], x2nT, k2T)
            proj_v(w["v2"], x2nT, v2)

            ctx2T = t2.tile([128, KD, S], f32r, tag="t2")
            attention(q2T, k2T, v2, ctx2T)
            if opts.get("ablate_att", False):
                ctx2T = q2T
            src2T = t2.tile([128, KD, S], f32r, tag="t2")
            att_out_T(ctx2T, boeT[2], x2nT, src2T)

            q12T = t2.tile([128, KD, S], f32r, tag="t2")
            k12T = t2.tile([128, KD, S], f32r, tag="t2")
            v12 = big8.tile([128, MT, D], f32r, tag="big8")
            proj_T(w["q12"], bqT["q12"], src1T, q12T)
            proj_T(w["k12"], bqT["k12"], src2T, k12T)
            proj_v(w["v12"], src2T, v12)

            ctx3T = t2.tile([128, KD, S], f32r, tag="t2")
            attention(q12T, k12T, v12, ctx3T)
            if opts.get("ablate_att", False):
                ctx3T = q12T
            x_sb = xsb_p.tile([128, MT, D], f32, tag="xsb")
            att_out_N(ctx3T, x1[b], x_sb)

            hT = t2.tile([128, KD, S], f32r, tag="t2")
            ln_rows_to_T(
                lambda mt: x_sb[:, mt, :], lnT["lnf_g"], lnT["lnf_b"], hT
            )
            # fold the final-residual bias b2 into x_sb (read by mlp2 below)
            for mt in range(MT):
                nc.vector.tensor_add(x_sb[:, mt, :], x_sb[:, mt, :], b2_rep[:])

            w2t = [load_wk(w2, k, pool=wk2, tag="wk2") for k in range(KM)]
            for chunk in range(QN):
                g1c = big8.tile([128, KM, 512], f32r, tag="big8")
                for half in range(2):
                    w1t = [
                        load_wk(w1, k, pool=wk1, tag="wk1", c0=half * 512)
                        for k in range(KD)
                    ]
                    for ml in range(4):
                        m = half * 4 + ml
                        ps = pp.tile([128, 512], f32, tag="pp")
                        for k in range(KD):
                            nc.tensor.matmul(
                                ps[:], w1t[k][:, ts(ml, 128)], hT[:, k, ds(chunk * 512, 512)],
                                start=(k == 0), stop=(k == KD - 1),
                            )
                        nc.scalar.activation(
                            g1c[:, m, :], ps[:], AF.Gelu, bias=b1T[:, m:m + 1]
                        )
                for mtl in range(4):
                    mt = chunk * 4 + mtl
                    ps = pp.tile([128, 512], f32, tag="pp")
                    for k in range(KM):
                        nc.tensor.matmul(
                            ps[:], g1c[:, k, ts(mtl, 128)], w2t[k][:],
                            start=(k == 0), stop=(k == KM - 1),
                        )
                    o = obuf.tile([128, D], f32, tag="ob")
                    nc.vector.tensor_add(o[:], ps[:], x_sb[:, mt, :])
                    nc.sync.dma_start(out[b, ts(mt, 128), :], o[:])

    return nc


_NC = None


def make_in_maps(inputs):
    f64 = np.float64
    Wo = inputs["Wo"].astype(f64)
    bo = inputs["bo"].astype(f64)
    boe = {
        1: (bo + inputs["bv1"].astype(f64) @ Wo).astype(np.float32),
        2: (bo + inputs["bv2"].astype(f64) @ Wo).astype(np.float32),
        3: (bo + inputs["bv12"].astype(f64) @ Wo).astype(np.float32),
    }
    common = {
        "ident": np.eye(128, dtype=np.float32),
        "ones_sq": np.ones((128, 128), dtype=np.float32),
        "boe1": boe[1], "boe2": boe[2], "boe3": boe[3],
    }
    for nm in ["Wq1", "Wk1", "Wv1", "Wq2", "Wk2", "Wv2", "Wq12", "Wk12", "Wv12",
               "Wo", "W1", "W2", "bq1", "bk1", "bq2", "bk2", "bq12", "bk12",
               "b1", "b2", "ln1_g", "ln1_b", "ln2_g", "ln2_b", "lnf_g", "lnf_b"]:
        common[nm] = np.ascontiguousarray(np.asarray(inputs[nm], dtype=np.float32))

    x1f = np.asarray(inputs["x1"], dtype=np.float32)
    x2f = np.asarray(inputs["x2"], dtype=np.float32)
    in_maps = []
    for c in range(NCORES):
        m = dict(common)
        m["x1"] = np.ascontiguousarray(x1f[c * BL:(c + 1) * BL])
        m["x2"] = np.ascontiguousarray(x2f[c * BL:(c + 1) * BL])
        in_maps.append(m)
    return in_maps


def kernel(**inputs):
    global _NC
    if _NC is None:
        _NC = _build()
    nc = _NC
    in_maps = make_in_maps(inputs)
    res = run_bass_kernel_spmd(nc, in_maps, list(range(NCORES))).results
    return np.concatenate([res[c]["out"] for c in range(NCORES)], axis=0)


if __name__ == "__main__":
    rng = np.random.default_rng(0)
    ins = {
        "x1": rng.standard_normal((B, S, D), dtype=np.float32),
        "x2": rng.standard_normal((B, S, D), dtype=np.float32),
    }
    for nm in ["q1", "k1", "v1", "q2", "k2", "v2", "q12", "k12", "v12", "o"]:
        ins["W" + nm] = rng.standard_normal((D, D), dtype=np.float32) * 0.02
        ins["b" + nm] = np.zeros(D, np.float32)
    ins["W1"] = rng.standard_normal((D, MLP), dtype=np.float32) * 0.02
    ins["b1"] = np.zeros(MLP, np.float32)
    ins["W2"] = rng.standard_normal((MLP, D), dtype=np.float32) * 0.02
    ins["b2"] = np.zeros(D, np.float32)
    for nm in ["ln1_g", "ln2_g", "lnf_g"]:
        ins[nm] = np.ones(D, np.float32)
    for nm in ["ln1_b", "ln2_b", "lnf_b"]:
        ins[nm] = np.zeros(D, np.float32)
    o = kernel(**ins)
    print("out", o.shape, o.dtype, float(np.abs(o).max()))

